# revision 13
# baseline (speedup 1.0000x reference)
"""Causal self-attention (S=2048, D=1024, H=16) on 8 Trainium2 NeuronCores.

Sharding: tensor-parallel over heads. Core c owns heads 2c, 2c+1:
  - computes qT/kT/vT for its 128 qkv-columns from the full hidden_states
    (contraction layouts; vT is PE-transposed back to natural [s, j]),
  - runs causal attention for its 2 heads (attT = K.Q^T blocks, exp via
    ScalarE, denominators via a ones-column in the PV matmul),
  - projects each head against its W_proj row-slice and fuses the softmax
    normalization into the projection epilogue (per-partition 1/den scales
    via DVE reciprocal on packed denominator rows + K-selector broadcast
    matmuls),
  - outputs a partial [S, D] bf16 product; the host sums the 8 partials
    and adds b_proj.

Engine budget notes (vs the 137us baseline):
  - no ScalarE Ln/Exp reciprocal -> single exp ACT table load (was 9)
  - den reciprocals on DVE in 3 batched calls, hidden under p1/p3 work
  - input DMAs issued as few large descriprtor batches (3D dram tensors)
  - no warmup dummy matmuls; p1(0) itself warms the PE clock
  - elementwise work spread across DVE / ScalarE / GpSimd
  - bf16 output partials (halves output DMA)
"""

import math
from contextlib import ExitStack

import numpy as np

import concourse.bacc as bacc
import concourse.mybir as mybir
import concourse.tile as tile
from concourse.bass_utils import run_bass_kernel_spmd

S, D, H = 2048, 1024, 16
HS = D // H  # 64 head size
P = 128
NCORES = 8
HPC = H // NCORES  # 2 heads per core
CD = HPC * HS  # 128 per-core head dims
KO = D // P  # 8 contraction tiles for the projections
NQC = S // 512  # 4 query chunks
NSC = S // P  # 16 sequence chunks of 128
SCALE = 1.0 / math.sqrt(S)

F32 = mybir.dt.float32
F32R = mybir.dt.float32r
BF16 = mybir.dt.bfloat16

try:
    import ml_dtypes

    NP_BF16 = ml_dtypes.bfloat16
except ImportError:  # pragma: no cover
    NP_BF16 = None


def _build():
    nc = bacc.Bacc(
        "TRN2", target_bir_lowering=False, debug=False, num_devices=NCORES
    )

    # hsT / w_qkv declared 3D so one DMA covers all 8 contraction chunks
    hsT = nc.dram_tensor("hsT", [KO, P, S], BF16, kind="ExternalInput")
    w_qkv = nc.dram_tensor("w_qkv", [KO, P, 3 * P], BF16, kind="ExternalInput")
    b_qkv = nc.dram_tensor("b_qkv", [P, 3], F32, kind="ExternalInput")
    w_p = nc.dram_tensor("w_p", [CD, D], F32R, kind="ExternalInput")
    cpack = nc.dram_tensor("cpack", [P, P + 896], BF16, kind="ExternalInput")
    vones = nc.dram_tensor("vones", [P, NSC], BF16, kind="ExternalInput")
    sel = nc.dram_tensor("sel", [4, 404], F32R, kind="ExternalInput")
    out = nc.dram_tensor("out", [S, D], BF16, kind="ExternalOutput")

    with (
        tile.TileContext(nc) as tc,
        ExitStack() as ctx,
        nc.allow_low_precision(reason="bf16/float32r matmul pipeline"),
    ):
        const = ctx.enter_context(tc.tile_pool(name="const", bufs=1))
        work = ctx.enter_context(tc.tile_pool(name="work", bufs=2))
        pp = ctx.enter_context(tc.tile_pool(name="pp", bufs=1, space="PSUM"))

        def psA(name):  # generic 2-bank matmul target, 3 slots
            return pp.tile([P, 2, 512], F32, tag="A", bufs=3, name=name)

        # ---- loads: few large descriptor batches ------------------------
        cpack_sb = const.tile([P, P + 896], BF16, tag="cpack", name="cpack_sb")
        nc.sync.dma_start(out=cpack_sb, in_=cpack.ap())
        identb = cpack_sb[:, 0:P]
        msk_sb = cpack_sb[:, P : P + 896]
        bqkv_sb = const.tile([P, 3], F32, tag="bqkv", name="bqkv_sb")
        nc.sync.dma_start(out=bqkv_sb, in_=b_qkv.ap())
        sel_sb = const.tile([4, 404], F32R, tag="sel", name="sel_sb")
        nc.sync.dma_start(out=sel_sb, in_=sel.ap())

        # hsT in three tiles so each batched DMA writes a full tile (walrus
        # rejects partition-interior dest APs when an outer dim is sliced)
        hsT_t = [
            const.tile([P, KO, 512], BF16, tag="hsT0", name="hsT0_sb"),
            const.tile([P, KO, 512], BF16, tag="hsT1", name="hsT1_sb"),
            const.tile([P, KO, 1024], BF16, tag="hsT23", name="hsT23_sb"),
        ]

        def hs_chunk(o, n):  # [P, 512] rhs slice for contraction tile o
            if n < 2:
                return hsT_t[n][:, o, :]
            return hsT_t[2][:, o, (n - 2) * 512 : (n - 1) * 512]

        wqkv_sb = const.tile([P, KO, 3 * P], BF16, tag="wqkv", name="wqkv_sb")
        nc.gpsimd.dma_start(
            out=wqkv_sb, in_=w_qkv.ap().rearrange("o p f -> p o f")
        )
        nc.sync.dma_start(
            out=hsT_t[0],
            in_=hsT.ap()[:, :, 0:512].rearrange("o p f -> p o f"),
        )
        v_sb = []
        for h in range(HPC):
            vt = const.tile([P, NSC, HS + 1], BF16, tag=f"v{h}", name=f"v{h}_sb")
            nc.gpsimd.dma_start(out=vt[:, :, HS], in_=vones.ap())
            v_sb.append(vt)
        wp_sb = const.tile([P, D], F32R, tag="wp", name="wp_sb")
        nc.sync.dma_start(out=wp_sb, in_=w_p.ap())
        nc.gpsimd.dma_start(
            out=hsT_t[1],
            in_=hsT.ap()[:, :, 512:1024].rearrange("o p f -> p o f"),
        )
        nc.gpsimd.dma_start(
            out=hsT_t[2],
            in_=hsT.ap()[:, :, 1024:2048].rearrange("o p f -> p o f"),
        )

        qkT_sb = const.tile([P, 2, S], BF16, tag="qkT", name="qkT_sb")
        vT_sb = const.tile([P, S], BF16, tag="vT", name="vT_sb")
        u2_sb = [
            const.tile([P, 512], F32R, tag=f"u2_{qc}", name=f"u2_{qc}")
            for qc in range(NQC)
        ]
        u2n_sb = [
            const.tile([P, 512], F32R, tag=f"u2n_{qc}", name=f"u2n_{qc}")
            for qc in range(NQC)
        ]
        # softmax denominator rows (engine APs must start at partition 0,
        # so each lives in its own [1, 512] tile; K=1 matmuls pack them)
        den_sb = {
            (qc, h): const.tile(
                [1, 512], F32R, tag=f"den_{qc}_{h}", name=f"den_{qc}_{h}"
            )
            for qc in range(NQC)
            for h in range(HPC)
        }
        rr01 = const.tile([4, 512], F32R, tag="rr01", name="rr01")
        rr2 = const.tile([2, 512], F32R, tag="rr2", name="rr2")
        rr3 = const.tile([2, 512], F32R, tag="rr3", name="rr3")

        # ---- phase 1: qT, kT, vT ([j, s] layout) + v transposes ----------
        def emit_p1(n):
            for m in range(3):
                ps_qkv = psA("ps_qkv")[:, 0, :]
                for o in range(KO):
                    nc.tensor.matmul(
                        ps_qkv,
                        lhsT=wqkv_sb[:, o, m * P : (m + 1) * P],
                        rhs=hs_chunk(o, n),
                        start=(o == 0),
                        stop=(o == KO - 1),
                    )
                dst = (
                    qkT_sb[:, m, n * 512 : (n + 1) * 512]
                    if m < 2
                    else vT_sb[:, n * 512 : (n + 1) * 512]
                )
                if m == 0:
                    nc.vector.tensor_scalar_add(
                        out=dst, in0=ps_qkv, scalar1=bqkv_sb[:, m : m + 1]
                    )
                else:
                    nc.scalar.activation(
                        out=dst,
                        in_=ps_qkv,
                        func=mybir.ActivationFunctionType.Identity,
                        bias=bqkv_sb[:, m : m + 1],
                    )
            # transpose this n-chunk of vT into natural v layout
            for sc in range(4 * n, 4 * n + 4):
                ps_t = pp.tile([P, P], BF16, tag="A", bufs=3, name="ps_t")
                nc.tensor.transpose(ps_t, vT_sb[:, sc * P : (sc + 1) * P], identb)
                for h in range(HPC):
                    nc.vector.tensor_copy(
                        out=v_sb[h][:, sc, 0:HS], in_=ps_t[:, h * HS : (h + 1) * HS]
                    )

        emit_p1(0)

        # ---- softmax normalization: pack den rows into PSUM with K=1
        # one-hot matmuls, 1/den via one DVE reciprocal per batch, then
        # K-selector matmuls broadcast the rows to the 128 u2 partitions.
        def emit_norm(which):
            rb = psA("ps_rb")
            if which == 0:  # qc 0 and 1 together
                rows = [(0, 0), (0, 1), (1, 0), (1, 1)]
                rr = rr01
            else:  # qc == which (2 or 3)
                rows = [(which, 0), (which, 1)]
                rr = rr2 if which == 2 else rr3
            nr = len(rows)
            base = 384 if nr == 4 else 400
            for r, (qc_r, h_r) in enumerate(rows):
                nc.tensor.matmul(
                    rb[0:nr, 0, :],
                    lhsT=sel_sb[0:1, base + nr * r : base + nr * (r + 1)],
                    rhs=den_sb[(qc_r, h_r)],
                    start=(r == 0),
                    stop=(r == nr - 1),
                )
            nc.vector.reciprocal(out=rr, in_=rb[0:nr, 0, :])
            if which == 0:
                for q in (0, 1):
                    nc.tensor.matmul(
                        rb[:, q, :],
                        lhsT=sel_sb[:, q * P : (q + 1) * P],
                        rhs=rr,
                        start=True,
                        stop=True,
                    )
                for q in (0, 1):
                    nc.vector.tensor_mul(
                        out=u2n_sb[q], in0=u2_sb[q], in1=rb[:, q, :]
                    )
            else:
                nc.tensor.matmul(
                    rb[:, 1, :],
                    lhsT=sel_sb[0:2, 256:384],
                    rhs=rr,
                    start=True,
                    stop=True,
                )
                nc.vector.tensor_mul(
                    out=u2n_sb[which], in0=u2_sb[which], in1=rb[:, 1, :]
                )

        # ---- phase 3: projection over both heads (K=128), pipelined ------
        def emit_p3_mm(sc):
            qc = sc // 4
            f = sc % 4
            slot = psA("ps_p3")
            for dc in range(2):
                nc.tensor.matmul(
                    slot[:, dc, :],
                    lhsT=u2n_sb[qc][:, f * P : (f + 1) * P],
                    rhs=wp_sb[:, dc * 512 : (dc + 1) * 512],
                    start=True,
                    stop=True,
                )
            return slot

        # sc 6..11 copy on ScalarE so the tail DVE queue stays clear for
        # the norm(3) reciprocal; everything else alternates by parity.
        def emit_p3_epi(sc, slot):
            out_t = work.tile([P, 2, 512], BF16, tag="out", bufs=3, name="out_t")
            on_vector = sc % 2 == 0 if sc < 6 else not (6 <= sc < 12)
            if on_vector:
                nc.vector.tensor_copy(out=out_t, in_=slot)
            else:
                nc.scalar.copy(out=out_t, in_=slot)
            eng = nc.sync if sc % 2 == 0 else nc.gpsimd
            eng.dma_start(
                out=out.ap()[sc * P : (sc + 1) * P, :],
                in_=out_t.rearrange("p a b -> p (a b)"),
            )

        p3q = []

        def p3_push(sc):
            p3q.append((sc, emit_p3_mm(sc)))
            if len(p3q) > 1:
                emit_p3_epi(*p3q.pop(0))

        # ---- phase 2: causal attention, software-pipelined ---------------
        for qc in range(NQC):
            ps_o = [
                pp.tile([P, 512], F32, tag="O", bufs=2, name=f"ps_o{h}")
                for h in range(HPC)
            ]
            nkb = 4 * (qc + 1)  # 128-wide key blocks in the causal span
            ngrp = nkb // 2

            def emit_pv(pend, nkb=nkb, ps_o=ps_o):
                pes, kbs, f0 = pend
                for h in range(HPC):
                    for j, kb in enumerate(kbs):
                        nc.tensor.matmul(
                            ps_o[h][0 : HS + 1, f0:512],
                            lhsT=v_sb[h][:, kb, :],
                            rhs=pes[h][:, j, f0:512],
                            start=(kb == 0),
                            stop=(kb == nkb - 1),
                        )

            pending = None  # exp'd logits awaiting their PV matmuls
            for g in range(ngrp):
                kbs = [2 * g, 2 * g + 1]
                # last group covers only the causal upper half of the q range
                f0 = 256 if g == ngrp - 1 else 0
                # logits for both heads, adjacent for row-group packing
                ps_att = [psA(f"ps_att{h}") for h in range(HPC)]
                for j, kb in enumerate(kbs):
                    for h in range(HPC):
                        nc.tensor.matmul(
                            ps_att[h][:, j, f0:512],
                            lhsT=qkT_sb[h * HS : (h + 1) * HS, 1, kb * P : (kb + 1) * P],
                            rhs=qkT_sb[h * HS : (h + 1) * HS, 0, qc * 512 + f0 : (qc + 1) * 512],
                            start=True,
                            stop=True,
                        )
                if pending is not None:
                    emit_pv(pending)
                pes = []
                for h in range(HPC):
                    p_exp = work.tile(
                        [P, 2, 512], BF16, tag=f"pe{h}", bufs=4, name="p_exp"
                    )
                    nc.scalar.activation(
                        out=p_exp[:, :, f0:512],
                        in_=ps_att[h][:, :, f0:512],
                        func=mybir.ActivationFunctionType.Exp,
                        scale=SCALE,
                    )
                    for j, kb in enumerate(kbs):
                        jj = kb - 4 * qc
                        if jj >= 0:  # diagonal block: causal 0/1 mask
                            off = 384 - 128 * jj
                            eng = nc.vector if h == 0 else nc.gpsimd
                            eng.tensor_mul(
                                out=p_exp[:, j, f0:512],
                                in0=p_exp[:, j, f0:512],
                                in1=msk_sb[:, off + f0 : off + 512],
                            )
                    pes.append(p_exp)
                pending = (pes, kbs, f0)
            emit_pv(pending)

            # stash unnormalized head outputs (ScalarE) + denominator rows
            for h in range(HPC):
                nc.scalar.copy(out=u2_sb[qc][h * HS : (h + 1) * HS, :], in_=ps_o[h][0:HS, :])
                nc.vector.tensor_copy(
                    out=den_sb[(qc, h)], in_=ps_o[h][HS : HS + 1, :]
                )
            if qc + 1 < NQC:
                emit_p1(qc + 1)
            if qc == 1:
                emit_norm(0)  # qc 0+1 together, hidden under p1(2)
            if qc == 2:
                for sc_e in (0, 1, 2, 3):
                    p3_push(sc_e)
                while p3q:  # don't carry PSUM slots into qc=3's rotation
                    emit_p3_epi(*p3q.pop(0))
                emit_norm(2)  # recip hidden under qc=3 groups

        # ---- tail: remaining projections; norm(3) overlaps sc 4..11 ------
        p3_push(4)
        p3_push(5)
        emit_norm(3)
        for sc in range(6, NSC):
            p3_push(sc)
        for item in p3q:
            emit_p3_epi(*item)

    nc.compile()
    return nc


_NC = None


def _get_nc():
    global _NC
    if _NC is None:
        _NC = _build()
    return _NC


def prepare_inputs(hidden_states, W_attn, b_attn, W_proj, b_proj):
    hs = np.asarray(hidden_states, dtype=np.float32)
    Wa = np.asarray(W_attn, dtype=np.float32)
    ba = np.asarray(b_attn, dtype=np.float32)
    Wp = np.asarray(W_proj, dtype=np.float32)

    hsT = np.ascontiguousarray(hs.T).astype(NP_BF16).reshape(KO, P, S)
    pcol = np.arange(P)[:, None]
    ccol = np.arange(896)[None, :]
    msk = (pcol <= ccol - 384).astype(NP_BF16)
    cpack = np.concatenate([np.eye(P).astype(NP_BF16), msk], axis=1)
    cpack = np.ascontiguousarray(cpack)

    # K-selector for the 1/den broadcast matmuls:
    #  cols   0:128 -> batch01, qc=0 half   (row k hot iff k == m//64)
    #  cols 128:256 -> batch01, qc=1 half   (row k hot iff k == 2 + m//64)
    #  cols 256:384 -> single-qc pairs      (row k hot iff k == m//64, k<2)
    sel = np.zeros((4, 404), dtype=np.float32)
    m = np.arange(P)
    sel[m // 64, m] = 1.0
    sel[2 + m // 64, 128 + m] = 1.0
    sel[m // 64, 256 + m] = 1.0
    for r in range(4):  # one-hot rows for the den pack matmuls
        sel[0, 384 + 4 * r + r] = 1.0
    for r in range(2):
        sel[0, 400 + 2 * r + r] = 1.0

    in_maps = []
    for c in range(NCORES):
        q0 = c * CD
        wq = Wa[:, q0 : q0 + CD]
        wk = Wa[:, D + q0 : D + q0 + CD]
        wv = Wa[:, 2 * D + q0 : 2 * D + q0 + CD]
        bq = ba[q0 : q0 + CD]
        bk = ba[D + q0 : D + q0 + CD]
        bv = ba[2 * D + q0 : 2 * D + q0 + CD]
        in_maps.append(
            {
                "hsT": hsT,
                "w_qkv": np.ascontiguousarray(
                    np.concatenate([wq, wk, wv], axis=1)
                ).astype(NP_BF16).reshape(KO, P, 3 * P),
                "b_qkv": np.ascontiguousarray(np.stack([bq, bk, bv], axis=1)).astype(
                    np.float32
                ),
                "w_p": np.ascontiguousarray(Wp[q0 : q0 + CD, :], dtype=np.float32),
                "cpack": cpack,
                "vones": np.ones((P, NSC)).astype(NP_BF16),
                "sel": sel,
            }
        )
    return in_maps


def run(inputs, trace=False):
    """Build+run the sharded kernel. Returns (full_output, BassKernelResults)."""
    in_maps = prepare_inputs(**inputs)
    nc = _get_nc()
    res = run_bass_kernel_spmd(
        nc, in_maps, core_ids=list(range(NCORES)), trace=trace
    )
    acc = np.zeros((S, D), dtype=np.float32)
    for c in range(NCORES):
        acc += np.asarray(res.results[c]["out"], dtype=np.float32)
    acc += np.asarray(inputs["b_proj"], dtype=np.float32)
    return acc, res


def kernel(**inputs):
    out, _ = run(inputs, trace=False)
    return out


# revision 15
# speedup vs baseline: 1.0791x; 1.0791x over previous
"""Causal self-attention (S=2048, D=1024, H=16) on 8 Trainium2 NeuronCores.

Sharding: tensor-parallel over heads. Core c owns heads 2c, 2c+1:
  - computes qT/kT/vT for its 128 qkv-columns from the full hidden_states
    (contraction layouts; vT is PE-transposed back to natural [s, j]),
  - runs causal attention for its 2 heads (attT = K.Q^T blocks, exp via
    ScalarE, denominators via a ones-column in the PV matmul),
  - projects each head against its W_proj row-slice and fuses the softmax
    normalization into the projection epilogue (per-partition 1/den scales
    via DVE reciprocal on packed denominator rows + K-selector broadcast
    matmuls),
  - outputs a partial [S, D] bf16 product; the host sums the 8 partials
    and adds b_proj.

Engine budget notes (vs the 137us baseline):
  - no ScalarE Ln/Exp reciprocal -> single exp ACT table load (was 9)
  - den reciprocals on DVE in 3 batched calls, hidden under p1/p3 work
  - input DMAs issued as few large descriprtor batches (3D dram tensors)
  - no warmup dummy matmuls; p1(0) itself warms the PE clock
  - elementwise work spread across DVE / ScalarE / GpSimd
  - bf16 output partials (halves output DMA)
"""

import math
from contextlib import ExitStack

import numpy as np

import concourse.bacc as bacc
import concourse.mybir as mybir
import concourse.tile as tile
from concourse.bass_utils import run_bass_kernel_spmd

S, D, H = 2048, 1024, 16
HS = D // H  # 64 head size
P = 128
NCORES = 8
HPC = H // NCORES  # 2 heads per core
CD = HPC * HS  # 128 per-core head dims
KO = D // P  # 8 contraction tiles for the projections
NQC = S // 512  # 4 query chunks
NSC = S // P  # 16 sequence chunks of 128
SCALE = 1.0 / math.sqrt(S)

F32 = mybir.dt.float32
F32R = mybir.dt.float32r
BF16 = mybir.dt.bfloat16

try:
    import ml_dtypes

    NP_BF16 = ml_dtypes.bfloat16
except ImportError:  # pragma: no cover
    NP_BF16 = None


def _build():
    nc = bacc.Bacc(
        "TRN2", target_bir_lowering=False, debug=False, num_devices=NCORES
    )

    # hsT / w_qkv declared 3D so one DMA covers all 8 contraction chunks
    hsT = nc.dram_tensor("hsT", [KO, P, S], BF16, kind="ExternalInput")
    w_qkv = nc.dram_tensor("w_qkv", [KO, P, 3 * P], BF16, kind="ExternalInput")
    b_qkv = nc.dram_tensor("b_qkv", [P, 3], F32, kind="ExternalInput")
    w_p = nc.dram_tensor("w_p", [CD, D], F32R, kind="ExternalInput")
    cpack = nc.dram_tensor("cpack", [P, P + 896], BF16, kind="ExternalInput")
    vones = nc.dram_tensor("vones", [P, NSC], BF16, kind="ExternalInput")
    sel = nc.dram_tensor("sel", [4, 404], F32R, kind="ExternalInput")
    out = nc.dram_tensor("out", [S, D], BF16, kind="ExternalOutput")

    with (
        tile.TileContext(nc) as tc,
        ExitStack() as ctx,
        nc.allow_low_precision(reason="bf16/float32r matmul pipeline"),
    ):
        const = ctx.enter_context(tc.tile_pool(name="const", bufs=1))
        work = ctx.enter_context(tc.tile_pool(name="work", bufs=2))
        pp = ctx.enter_context(tc.tile_pool(name="pp", bufs=1, space="PSUM"))

        def psA(name):  # generic 2-bank matmul target, 3 slots
            return pp.tile([P, 2, 512], F32, tag="A", bufs=3, name=name)

        # ---- loads: fine-grained per-o streaming, priority order ---------
        # (p1(0)'s inputs first so the PE starts ~9us in; later chunks
        # stream behind while attention runs)
        hsT_t = [
            const.tile([P, KO, 512], BF16, tag="hsT0", name="hsT0_sb"),
            const.tile([P, KO, 512], BF16, tag="hsT1", name="hsT1_sb"),
            const.tile([P, KO, 1024], BF16, tag="hsT23", name="hsT23_sb"),
        ]

        def hs_chunk(o, n):  # [P, 512] rhs slice for contraction tile o
            if n < 2:
                return hsT_t[n][:, o, :]
            return hsT_t[2][:, o, (n - 2) * 512 : (n - 1) * 512]

        wqkv_sb = const.tile([P, KO, 3 * P], BF16, tag="wqkv", name="wqkv_sb")
        for o in range(KO):
            nc.sync.dma_start(
                out=hsT_t[0][:, o, :], in_=hsT.ap()[o, :, 0:512]
            )
            nc.gpsimd.dma_start(out=wqkv_sb[:, o, :], in_=w_qkv.ap()[o])
        bqkv_sb = const.tile([P, 3], F32, tag="bqkv", name="bqkv_sb")
        nc.sync.dma_start(out=bqkv_sb, in_=b_qkv.ap())
        cpack_sb = const.tile([P, P + 896], BF16, tag="cpack", name="cpack_sb")
        nc.sync.dma_start(out=cpack_sb, in_=cpack.ap())
        identb = cpack_sb[:, 0:P]
        msk_sb = cpack_sb[:, P : P + 896]
        sel_sb = const.tile([4, 404], F32R, tag="sel", name="sel_sb")
        nc.sync.dma_start(out=sel_sb, in_=sel.ap())
        v_sb = []
        for h in range(HPC):
            vt = const.tile([P, NSC, HS + 1], BF16, tag=f"v{h}", name=f"v{h}_sb")
            nc.gpsimd.dma_start(out=vt[:, :, HS], in_=vones.ap())
            v_sb.append(vt)
        wp_sb = const.tile([P, D], F32R, tag="wp", name="wp_sb")
        nc.sync.dma_start(out=wp_sb, in_=w_p.ap())
        for o in range(KO):
            nc.sync.dma_start(
                out=hsT_t[1][:, o, :], in_=hsT.ap()[o, :, 512:1024]
            )
            nc.gpsimd.dma_start(
                out=hsT_t[2][:, o, :], in_=hsT.ap()[o, :, 1024:2048]
            )

        qkT_sb = const.tile([P, 2, S], BF16, tag="qkT", name="qkT_sb")
        vT_sb = const.tile([P, S], BF16, tag="vT", name="vT_sb")
        u2_sb = [
            const.tile([P, 512], F32R, tag=f"u2_{qc}", name=f"u2_{qc}")
            for qc in range(NQC)
        ]
        u2n_sb = [
            const.tile([P, 512], F32R, tag=f"u2n_{qc}", name=f"u2n_{qc}")
            for qc in range(NQC)
        ]
        # softmax denominator rows (engine APs must start at partition 0,
        # so each lives in its own [1, 512] tile; K=1 matmuls pack them)
        den_sb = {
            (qc, h): const.tile(
                [1, 512], F32R, tag=f"den_{qc}_{h}", name=f"den_{qc}_{h}"
            )
            for qc in range(NQC)
            for h in range(HPC)
        }
        rr01 = const.tile([4, 512], F32R, tag="rr01", name="rr01")
        rr2 = const.tile([2, 512], F32R, tag="rr2", name="rr2")
        rr3 = const.tile([2, 512], F32R, tag="rr3", name="rr3")

        # ---- phase 1: qT, kT, vT ([j, s] layout) + v transposes ----------
        def emit_p1(n):
            for m in range(3):
                ps_qkv = psA("ps_qkv")[:, 0, :]
                for o in range(KO):
                    nc.tensor.matmul(
                        ps_qkv,
                        lhsT=wqkv_sb[:, o, m * P : (m + 1) * P],
                        rhs=hs_chunk(o, n),
                        start=(o == 0),
                        stop=(o == KO - 1),
                    )
                dst = (
                    qkT_sb[:, m, n * 512 : (n + 1) * 512]
                    if m < 2
                    else vT_sb[:, n * 512 : (n + 1) * 512]
                )
                if m == 0:
                    nc.vector.tensor_scalar_add(
                        out=dst, in0=ps_qkv, scalar1=bqkv_sb[:, m : m + 1]
                    )
                else:
                    nc.scalar.activation(
                        out=dst,
                        in_=ps_qkv,
                        func=mybir.ActivationFunctionType.Identity,
                        bias=bqkv_sb[:, m : m + 1],
                    )
            # transpose this n-chunk of vT into natural v layout
            for sc in range(4 * n, 4 * n + 4):
                ps_t = pp.tile([P, P], BF16, tag="A", bufs=3, name="ps_t")
                nc.tensor.transpose(ps_t, vT_sb[:, sc * P : (sc + 1) * P], identb)
                for h in range(HPC):
                    nc.vector.tensor_copy(
                        out=v_sb[h][:, sc, 0:HS], in_=ps_t[:, h * HS : (h + 1) * HS]
                    )

        emit_p1(0)

        # ---- softmax normalization: pack den rows into PSUM with K=1
        # one-hot matmuls, 1/den via one DVE reciprocal per batch, then
        # K-selector matmuls broadcast the rows to the 128 u2 partitions.
        def emit_norm(which):
            rb = psA("ps_rb")
            if which == 0:  # qc 0 and 1 together
                rows = [(0, 0), (0, 1), (1, 0), (1, 1)]
                rr = rr01
            else:  # qc == which (2 or 3)
                rows = [(which, 0), (which, 1)]
                rr = rr2 if which == 2 else rr3
            nr = len(rows)
            base = 384 if nr == 4 else 400
            for r, (qc_r, h_r) in enumerate(rows):
                nc.tensor.matmul(
                    rb[0:nr, 0, :],
                    lhsT=sel_sb[0:1, base + nr * r : base + nr * (r + 1)],
                    rhs=den_sb[(qc_r, h_r)],
                    start=(r == 0),
                    stop=(r == nr - 1),
                )
            nc.vector.reciprocal(out=rr, in_=rb[0:nr, 0, :])
            if which == 0:
                for q in (0, 1):
                    nc.tensor.matmul(
                        rb[:, q, :],
                        lhsT=sel_sb[:, q * P : (q + 1) * P],
                        rhs=rr,
                        start=True,
                        stop=True,
                    )
                for q in (0, 1):
                    nc.vector.tensor_mul(
                        out=u2n_sb[q], in0=u2_sb[q], in1=rb[:, q, :]
                    )
            else:
                nc.tensor.matmul(
                    rb[:, 1, :],
                    lhsT=sel_sb[0:2, 256:384],
                    rhs=rr,
                    start=True,
                    stop=True,
                )
                nc.vector.tensor_mul(
                    out=u2n_sb[which], in0=u2_sb[which], in1=rb[:, 1, :]
                )

        # ---- phase 3: projection over both heads (K=128), pipelined ------
        def emit_p3_mm(sc):
            qc = sc // 4
            f = sc % 4
            slot = psA("ps_p3")
            for dc in range(2):
                nc.tensor.matmul(
                    slot[:, dc, :],
                    lhsT=u2n_sb[qc][:, f * P : (f + 1) * P],
                    rhs=wp_sb[:, dc * 512 : (dc + 1) * 512],
                    start=True,
                    stop=True,
                )
            return slot

        # epilogue copies mostly on DVE; ScalarE takes the ones that land
        # where its exp stream is idle (qc=2 boundary, tail during recip3).
        def emit_p3_epi(sc, slot):
            out_t = work.tile([P, 2, 512], BF16, tag="out", bufs=3, name="out_t")
            on_vector = sc not in (1, 3, 10, 11)
            if on_vector:
                nc.vector.tensor_copy(out=out_t, in_=slot)
            else:
                nc.scalar.copy(out=out_t, in_=slot)
            eng = nc.sync if sc % 2 == 0 else nc.gpsimd
            eng.dma_start(
                out=out.ap()[sc * P : (sc + 1) * P, :],
                in_=out_t.rearrange("p a b -> p (a b)"),
            )

        p3q = []

        def p3_push(sc):
            p3q.append((sc, emit_p3_mm(sc)))
            if len(p3q) > 1:
                emit_p3_epi(*p3q.pop(0))

        # ---- phase 2: causal attention, software-pipelined ---------------
        for qc in range(NQC):
            ps_o = [
                pp.tile([P, 512], F32, tag="O", bufs=2, name=f"ps_o{h}")
                for h in range(HPC)
            ]
            nkb = 4 * (qc + 1)  # 128-wide key blocks in the causal span
            ngrp = nkb // 2

            def emit_pv(pend, nkb=nkb, ps_o=ps_o):
                pes, kbs, f0 = pend
                for h in range(HPC):
                    for j, kb in enumerate(kbs):
                        nc.tensor.matmul(
                            ps_o[h][0 : HS + 1, f0:512],
                            lhsT=v_sb[h][:, kb, :],
                            rhs=pes[h][:, j, f0:512],
                            start=(kb == 0),
                            stop=(kb == nkb - 1),
                        )

            pending = None  # exp'd logits awaiting their PV matmuls
            for g in range(ngrp):
                kbs = [2 * g, 2 * g + 1]
                # last group covers only the causal upper half of the q range
                f0 = 256 if g == ngrp - 1 else 0
                # logits for both heads, adjacent for row-group packing
                ps_att = [psA(f"ps_att{h}") for h in range(HPC)]
                for j, kb in enumerate(kbs):
                    for h in range(HPC):
                        nc.tensor.matmul(
                            ps_att[h][:, j, f0:512],
                            lhsT=qkT_sb[h * HS : (h + 1) * HS, 1, kb * P : (kb + 1) * P],
                            rhs=qkT_sb[h * HS : (h + 1) * HS, 0, qc * 512 + f0 : (qc + 1) * 512],
                            start=True,
                            stop=True,
                        )
                if pending is not None:
                    emit_pv(pending)
                pes = []
                for h in range(HPC):
                    p_exp = work.tile(
                        [P, 2, 512], BF16, tag=f"pe{h}", bufs=4, name="p_exp"
                    )
                    nc.scalar.activation(
                        out=p_exp[:, :, f0:512],
                        in_=ps_att[h][:, :, f0:512],
                        func=mybir.ActivationFunctionType.Exp,
                        scale=SCALE,
                    )
                    for j, kb in enumerate(kbs):
                        jj = kb - 4 * qc
                        if jj >= 0:  # diagonal block: causal 0/1 mask
                            off = 384 - 128 * jj
                            eng = nc.vector if h == 0 else nc.gpsimd
                            eng.tensor_mul(
                                out=p_exp[:, j, f0:512],
                                in0=p_exp[:, j, f0:512],
                                in1=msk_sb[:, off + f0 : off + 512],
                            )
                    pes.append(p_exp)
                pending = (pes, kbs, f0)
                # qc=3 is exp-paced with the PE half idle: slot the tail
                # projections under it (A-slot rotation [att0, att1, p3])
                if qc == 3 and 1 <= g <= 6:
                    p3_push(3 + g)
            emit_pv(pending)

            # stash unnormalized head outputs + denominator rows (DVE)
            for h in range(HPC):
                nc.vector.tensor_copy(
                    out=u2_sb[qc][h * HS : (h + 1) * HS, :], in_=ps_o[h][0:HS, :]
                )
                nc.vector.tensor_copy(
                    out=den_sb[(qc, h)], in_=ps_o[h][HS : HS + 1, :]
                )
            if qc + 1 < NQC:
                emit_p1(qc + 1)
            if qc == 1:
                emit_norm(0)  # qc 0+1 together, hidden under p1(2)
            if qc == 2:
                for sc_e in (0, 1, 2, 3):
                    p3_push(sc_e)
                while p3q:  # don't carry PSUM slots into qc=3's rotation
                    emit_p3_epi(*p3q.pop(0))
                emit_norm(2)  # recip hidden under qc=3 groups

        # ---- tail: norm(3)'s reciprocal overlaps sc 10/11 ----------------
        p3_push(10)
        p3_push(11)
        emit_norm(3)
        for sc in range(12, NSC):
            p3_push(sc)
        for item in p3q:
            emit_p3_epi(*item)

    nc.compile()
    return nc


_NC = None


def _get_nc():
    global _NC
    if _NC is None:
        _NC = _build()
    return _NC


def prepare_inputs(hidden_states, W_attn, b_attn, W_proj, b_proj):
    hs = np.asarray(hidden_states, dtype=np.float32)
    Wa = np.asarray(W_attn, dtype=np.float32)
    ba = np.asarray(b_attn, dtype=np.float32)
    Wp = np.asarray(W_proj, dtype=np.float32)

    hsT = np.ascontiguousarray(hs.T).astype(NP_BF16).reshape(KO, P, S)
    pcol = np.arange(P)[:, None]
    ccol = np.arange(896)[None, :]
    msk = (pcol <= ccol - 384).astype(NP_BF16)
    cpack = np.concatenate([np.eye(P).astype(NP_BF16), msk], axis=1)
    cpack = np.ascontiguousarray(cpack)

    # K-selector for the 1/den broadcast matmuls:
    #  cols   0:128 -> batch01, qc=0 half   (row k hot iff k == m//64)
    #  cols 128:256 -> batch01, qc=1 half   (row k hot iff k == 2 + m//64)
    #  cols 256:384 -> single-qc pairs      (row k hot iff k == m//64, k<2)
    sel = np.zeros((4, 404), dtype=np.float32)
    m = np.arange(P)
    sel[m // 64, m] = 1.0
    sel[2 + m // 64, 128 + m] = 1.0
    sel[m // 64, 256 + m] = 1.0
    for r in range(4):  # one-hot rows for the den pack matmuls
        sel[0, 384 + 4 * r + r] = 1.0
    for r in range(2):
        sel[0, 400 + 2 * r + r] = 1.0

    in_maps = []
    for c in range(NCORES):
        q0 = c * CD
        wq = Wa[:, q0 : q0 + CD]
        wk = Wa[:, D + q0 : D + q0 + CD]
        wv = Wa[:, 2 * D + q0 : 2 * D + q0 + CD]
        bq = ba[q0 : q0 + CD]
        bk = ba[D + q0 : D + q0 + CD]
        bv = ba[2 * D + q0 : 2 * D + q0 + CD]
        in_maps.append(
            {
                "hsT": hsT,
                "w_qkv": np.ascontiguousarray(
                    np.concatenate([wq, wk, wv], axis=1)
                ).astype(NP_BF16).reshape(KO, P, 3 * P),
                "b_qkv": np.ascontiguousarray(np.stack([bq, bk, bv], axis=1)).astype(
                    np.float32
                ),
                "w_p": np.ascontiguousarray(Wp[q0 : q0 + CD, :], dtype=np.float32),
                "cpack": cpack,
                "vones": np.ones((P, NSC)).astype(NP_BF16),
                "sel": sel,
            }
        )
    return in_maps


def run(inputs, trace=False):
    """Build+run the sharded kernel. Returns (full_output, BassKernelResults)."""
    in_maps = prepare_inputs(**inputs)
    nc = _get_nc()
    res = run_bass_kernel_spmd(
        nc, in_maps, core_ids=list(range(NCORES)), trace=trace
    )
    acc = np.zeros((S, D), dtype=np.float32)
    for c in range(NCORES):
        acc += np.asarray(res.results[c]["out"], dtype=np.float32)
    acc += np.asarray(inputs["b_proj"], dtype=np.float32)
    return acc, res


def kernel(**inputs):
    out, _ = run(inputs, trace=False)
    return out
